# revision 5
# baseline (speedup 1.0000x reference)
"""AbstractBlast kernel for 8x TRN2 NeuronCores.

Math (per token row x_t of length 4096, b_in=b_out=16, bs=256, rank=128):
  y_j = x_j @ Vt_j                    (j = input block, Vt_j: 256x128)
  z_o = sum_j S[o,j,:] * y_j          (elementwise over rank)
  out_o = z_o @ U_o + bias_o          (U_o: 128x256)

Sharding: data-parallel over batch B=8 -> core b handles x[b] (1024 tokens).
Factors replicated. All on-chip matmuls keep tokens on the free dim, so x is
shipped pre-transposed (in_dim on partitions) and the output is computed
transposed and un-transposed on the host.
"""

import numpy as np

import concourse.bass as bass
import concourse.mybir as mybir
from concourse.bass_utils import run_bass_kernel_spmd
from concourse.tile import TileContext

F32 = mybir.dt.float32

B, T, D = 8, 1024, 4096
BIN, BOUT, BSIN, BSOUT, RANK = 16, 16, 256, 256, 128
NBLK = 2          # token blocks per core
NTOK = T // NBLK  # tokens per block (512)

_CACHE = {}


def _split_multi_waits(nc):
    """This walrus build encodes at most one sync wait per instruction, but
    Tile emits sync_info with several waits ("Too many sync wait commands").
    Split the extras into preceding same-engine NoOps with one wait each."""
    n_split = 0
    for fn in nc.m.functions:
        for bb in fn.blocks:
            new_insts = []
            for inst in bb.instructions:
                si = inst.sync_info
                if si is not None and si.on_wait and len(si.on_wait) > 1:
                    waits = list(si.on_wait)
                    for w in waits[:-1]:
                        nop = mybir.InstNoOp(
                            name=f"{inst.name}-wsplit-{n_split}",
                            ins=[],
                            outs=[],
                            engine=inst.engine,
                            sync_info=mybir.SyncInfo(on_wait=[w], on_update=[]),
                        )
                        n_split += 1
                        new_insts.append(nop)
                    inst.sync_info = mybir.SyncInfo(
                        on_wait=[waits[-1]], on_update=list(si.on_update)
                    )
                new_insts.append(inst)
            bb.instructions = new_insts
    return n_split


def _build_kernel():
    nc = bass.Bass(trn_type="TRN2")

    # Inputs (per core). xt: x[b].T reshaped (j, k, p, t) with in_dim = (j,k,p).
    xt = nc.dram_tensor("xt", [BIN, 128, 2, T], F32, kind="ExternalInput")
    # vt_w[p, j, k, r]: lhsT chunks; vt_w[:, j, k, :] = Vt[j][128k:128k+128, :]
    vt_w = nc.dram_tensor("vt_w", [128, BIN, 2, RANK], F32, kind="ExternalInput")
    # u_w[p, o, c]: lhsT for stage 3; u_w[:, o, 128h:128(h+1)] = U[o][:, 128h:...]
    u_w = nc.dram_tensor("u_w", [128, BOUT, BSOUT], F32, kind="ExternalInput")
    # s_w[p, o*16+j] = S[o, j, p]
    s_w = nc.dram_tensor("s_w", [128, BOUT * BIN], F32, kind="ExternalInput")
    # bias_w[p, 2o+h] = bias[o*256 + h*128 + p]
    bias_w = nc.dram_tensor("bias_w", [128, BOUT * 2], F32, kind="ExternalInput")
    # Output, transposed: ot[o, p, h, t] = out[t, o*256 + h*128 + p]
    ot = nc.dram_tensor("ot", [BOUT, 128, 2, T], F32, kind="ExternalOutput")

    mult = mybir.AluOpType.mult
    add = mybir.AluOpType.add

    with TileContext(nc) as tc:
        with (
            tc.tile_pool(name="wpool", bufs=1) as wpool,
            tc.tile_pool(name="xpool", bufs=6) as xpool,
            tc.tile_pool(name="ypool", bufs=2 * BIN + 2) as ypool,
            tc.tile_pool(name="zpool", bufs=4) as zpool,
            tc.tile_pool(name="opool", bufs=4) as opool,
            tc.tile_pool(name="ypsum", bufs=3, space="PSUM") as ypsum,
            tc.tile_pool(name="opsum", bufs=4, space="PSUM") as opsum,
        ):
            vt_t = wpool.tile([128, BIN, 2, RANK], F32)
            nc.sync.dma_start(out=vt_t, in_=vt_w[:, :, :, :])
            u_t = wpool.tile([128, BOUT, BSOUT], F32)
            nc.sync.dma_start(out=u_t, in_=u_w[:, :, :])
            s_t = wpool.tile([128, BOUT * BIN], F32)
            nc.sync.dma_start(out=s_t, in_=s_w[:, :])
            bias_t = wpool.tile([128, BOUT * 2], F32)
            nc.sync.dma_start(out=bias_t, in_=bias_w[:, :])

            for blk in range(NBLK):
                tok = slice(blk * NTOK, (blk + 1) * NTOK)

                # ---- stage 1: y_j = x_j @ Vt_j  (16 PSUM accum pairs) ----
                y_sb = []
                for j in range(BIN):
                    x_t = xpool.tile([128, 2, NTOK], F32, tag="xt")
                    nc.sync.dma_start(out=x_t, in_=xt[j, :, :, tok])
                    y_ps = ypsum.tile([128, NTOK], F32)
                    for k in range(2):
                        nc.tensor.matmul(
                            y_ps,
                            vt_t[:, j, k, :],
                            x_t[:, k, :],
                            start=(k == 0),
                            stop=(k == 1),
                        )
                    y = ypool.tile([128, NTOK], F32, tag="y")
                    nc.scalar.copy(y, y_ps)  # ACT eviction PSUM->SBUF
                    y_sb.append(y)

                # ---- stage 2 + 3 per output block o ----
                for o in range(BOUT):
                    z = zpool.tile([128, NTOK], F32, tag="z")
                    # j=0 term: scaled copy on ACT
                    nc.scalar.mul(z, y_sb[0], s_t[:, o * BIN : o * BIN + 1])
                    # j=1..15: fused MAC z = y_j * s + z
                    eng = nc.vector
                    for j in range(1, BIN):
                        eng.scalar_tensor_tensor(
                            z,
                            y_sb[j],
                            s_t[:, o * BIN + j : o * BIN + j + 1],
                            z,
                            mult,
                            add,
                        )
                    # stage 3: out_o = z @ U_o + bias
                    o_sb = opool.tile([128, 2, NTOK], F32, tag="o")
                    for h in range(2):
                        o_ps = opsum.tile([128, NTOK], F32)
                        nc.tensor.matmul(
                            o_ps,
                            u_t[:, o, 128 * h : 128 * (h + 1)],
                            z,
                            start=True,
                            stop=True,
                        )
                        nc.scalar.activation(
                            o_sb[:, h, :],
                            o_ps,
                            mybir.ActivationFunctionType.Identity,
                            bias=bias_t[:, 2 * o + h : 2 * o + h + 1],
                            scale=1.0,
                        )
                    nc.sync.dma_start(out=ot[o, :, :, tok], in_=o_sb)

    _split_multi_waits(nc)
    return nc


def kernel(x, S, U, Vt, bias):
    x = np.asarray(x, dtype=np.float32)
    S = np.asarray(S, dtype=np.float32)
    U = np.asarray(U, dtype=np.float32)
    Vt = np.asarray(Vt, dtype=np.float32)
    bias = np.asarray(bias, dtype=np.float32)

    # replicated factor layouts
    vt_w = np.ascontiguousarray(
        Vt.reshape(BIN, 2, 128, RANK).transpose(2, 0, 1, 3)
    )  # (128, j, k, r)
    u_w = np.ascontiguousarray(U.transpose(1, 0, 2))  # (128, o, c)
    s_w = np.ascontiguousarray(S.transpose(2, 0, 1).reshape(128, BOUT * BIN))
    bias_w = np.ascontiguousarray(
        bias.reshape(BOUT, 2, 128).transpose(2, 0, 1).reshape(128, BOUT * 2)
    )

    if "nc" not in _CACHE:
        _CACHE["nc"] = _build_kernel()
    nc = _CACHE["nc"]

    in_maps = []
    for b in range(B):
        xt = np.ascontiguousarray(
            x[b].T.reshape(BIN, 2, 128, T).transpose(0, 2, 1, 3)
        )
        in_maps.append(
            {"xt": xt, "vt_w": vt_w, "u_w": u_w, "s_w": s_w, "bias_w": bias_w}
        )

    res = run_bass_kernel_spmd(nc, in_maps, core_ids=list(range(B)))

    out = np.empty((B, T, D), dtype=np.float32)
    for b in range(B):
        o = res.results[b]["ot"]  # (o, p, h, t)
        out[b] = o.transpose(3, 0, 2, 1).reshape(T, D)
    return out


# revision 6
# speedup vs baseline: 1.0085x; 1.0085x over previous
"""AbstractBlast kernel for 8x TRN2 NeuronCores.

Math (per token row x_t of length 4096, b_in=b_out=16, bs=256, rank=128):
  y_j = x_j @ Vt_j                    (j = input block, Vt_j: 256x128)
  z_o = sum_j S[o,j,:] * y_j          (elementwise over rank)
  out_o = z_o @ U_o + bias_o          (U_o: 128x256)

Sharding: data-parallel over batch B=8 -> core b handles x[b] (1024 tokens).
Factors replicated. All on-chip matmuls keep tokens on the free dim, so x is
shipped pre-transposed (in_dim on partitions) and the output is computed
transposed and un-transposed on the host.
"""

import numpy as np

import concourse.bass as bass
import concourse.mybir as mybir
from concourse.bass_utils import run_bass_kernel_spmd
from concourse.tile import TileContext

F32 = mybir.dt.float32

B, T, D = 8, 1024, 4096
BIN, BOUT, BSIN, BSOUT, RANK = 16, 16, 256, 256, 128
NBLK = 2          # token blocks per core
NTOK = T // NBLK  # tokens per block (512)

_CACHE = {}


def _split_multi_waits(nc):
    """This walrus build encodes at most one sync wait per instruction, but
    Tile emits sync_info with several waits ("Too many sync wait commands").
    Split the extras into preceding same-engine NoOps with one wait each."""
    n_split = 0
    for fn in nc.m.functions:
        for bb in fn.blocks:
            new_insts = []
            for inst in bb.instructions:
                si = inst.sync_info
                if si is not None and si.on_wait and len(si.on_wait) > 1:
                    waits = list(si.on_wait)
                    for w in waits[:-1]:
                        nop = mybir.InstNoOp(
                            name=f"{inst.name}-wsplit-{n_split}",
                            ins=[],
                            outs=[],
                            engine=inst.engine,
                            sync_info=mybir.SyncInfo(on_wait=[w], on_update=[]),
                        )
                        n_split += 1
                        new_insts.append(nop)
                    inst.sync_info = mybir.SyncInfo(
                        on_wait=[waits[-1]], on_update=list(si.on_update)
                    )
                new_insts.append(inst)
            bb.instructions = new_insts
    return n_split


def _build_kernel(split_waits=True):
    nc = bass.Bass(trn_type="TRN2")

    # Inputs (per core). xt: x[b].T reshaped (j, k, p, t) with in_dim = (j,k,p).
    xt = nc.dram_tensor("xt", [BIN, 128, 2, T], F32, kind="ExternalInput")
    # vt_w[p, j, k, r]: lhsT chunks; vt_w[:, j, k, :] = Vt[j][128k:128k+128, :]
    vt_w = nc.dram_tensor("vt_w", [128, BIN, 2, RANK], F32, kind="ExternalInput")
    # u_w[p, o, c]: lhsT for stage 3; u_w[:, o, 128h:128(h+1)] = U[o][:, 128h:...]
    u_w = nc.dram_tensor("u_w", [128, BOUT, BSOUT], F32, kind="ExternalInput")
    # s_w[p, o*16+j] = S[o, j, p]
    s_w = nc.dram_tensor("s_w", [128, BOUT * BIN], F32, kind="ExternalInput")
    # bias_w[p, 2o+h] = bias[o*256 + h*128 + p]
    bias_w = nc.dram_tensor("bias_w", [128, BOUT * 2], F32, kind="ExternalInput")
    # Output, transposed: ot[o, p, h, t] = out[t, o*256 + h*128 + p]
    ot = nc.dram_tensor("ot", [BOUT, 128, 2, T], F32, kind="ExternalOutput")

    mult = mybir.AluOpType.mult
    add = mybir.AluOpType.add

    with TileContext(nc) as tc:
        with (
            tc.tile_pool(name="wpool", bufs=1) as wpool,
            tc.tile_pool(name="xpool", bufs=6) as xpool,
            tc.tile_pool(name="ypool", bufs=2 * BIN + 2) as ypool,
            tc.tile_pool(name="zpool", bufs=4) as zpool,
            tc.tile_pool(name="opool", bufs=4) as opool,
            tc.tile_pool(name="ypsum", bufs=3, space="PSUM") as ypsum,
            tc.tile_pool(name="opsum", bufs=4, space="PSUM") as opsum,
        ):
            vt_t = wpool.tile([128, BIN, 2, RANK], F32)
            nc.sync.dma_start(out=vt_t, in_=vt_w[:, :, :, :])
            u_t = wpool.tile([128, BOUT, BSOUT], F32)
            nc.sync.dma_start(out=u_t, in_=u_w[:, :, :])
            s_t = wpool.tile([128, BOUT * BIN], F32)
            nc.sync.dma_start(out=s_t, in_=s_w[:, :])
            bias_t = wpool.tile([128, BOUT * 2], F32)
            nc.sync.dma_start(out=bias_t, in_=bias_w[:, :])

            for blk in range(NBLK):
                tok = slice(blk * NTOK, (blk + 1) * NTOK)

                # ---- stage 1: y_j = x_j @ Vt_j  (16 PSUM accum pairs) ----
                y_sb = []
                for j in range(BIN):
                    x_t = xpool.tile([128, 2, NTOK], F32, tag="xt")
                    nc.sync.dma_start(out=x_t, in_=xt[j, :, :, tok])
                    y_ps = ypsum.tile([128, NTOK], F32)
                    for k in range(2):
                        nc.tensor.matmul(
                            y_ps,
                            vt_t[:, j, k, :],
                            x_t[:, k, :],
                            start=(k == 0),
                            stop=(k == 1),
                        )
                    y = ypool.tile([128, NTOK], F32, tag="y")
                    nc.scalar.copy(y, y_ps)  # ACT eviction PSUM->SBUF
                    y_sb.append(y)

                # ---- stage 2 + 3 per output block o ----
                for o in range(BOUT):
                    z = zpool.tile([128, NTOK], F32, tag="z")
                    # j=0 term: scaled copy on ACT
                    nc.scalar.mul(z, y_sb[0], s_t[:, o * BIN : o * BIN + 1])
                    # j=1..15: fused MAC z = y_j * s + z
                    eng = nc.vector
                    for j in range(1, BIN):
                        eng.scalar_tensor_tensor(
                            z,
                            y_sb[j],
                            s_t[:, o * BIN + j : o * BIN + j + 1],
                            z,
                            mult,
                            add,
                        )
                    # stage 3: out_o = z @ U_o + bias
                    o_sb = opool.tile([128, 2, NTOK], F32, tag="o")
                    for h in range(2):
                        o_ps = opsum.tile([128, NTOK], F32)
                        nc.tensor.matmul(
                            o_ps,
                            u_t[:, o, 128 * h : 128 * (h + 1)],
                            z,
                            start=True,
                            stop=True,
                        )
                        nc.scalar.activation(
                            o_sb[:, h, :],
                            o_ps,
                            mybir.ActivationFunctionType.Identity,
                            bias=bias_t[:, 2 * o + h : 2 * o + h + 1],
                            scale=1.0,
                        )
                    nc.sync.dma_start(out=ot[o, :, :, tok], in_=o_sb)

    if split_waits:
        _split_multi_waits(nc)
    return nc


def kernel(x, S, U, Vt, bias):
    x = np.asarray(x, dtype=np.float32)
    S = np.asarray(S, dtype=np.float32)
    U = np.asarray(U, dtype=np.float32)
    Vt = np.asarray(Vt, dtype=np.float32)
    bias = np.asarray(bias, dtype=np.float32)

    # replicated factor layouts
    vt_w = np.ascontiguousarray(
        Vt.reshape(BIN, 2, 128, RANK).transpose(2, 0, 1, 3)
    )  # (128, j, k, r)
    u_w = np.ascontiguousarray(U.transpose(1, 0, 2))  # (128, o, c)
    s_w = np.ascontiguousarray(S.transpose(2, 0, 1).reshape(128, BOUT * BIN))
    bias_w = np.ascontiguousarray(
        bias.reshape(BOUT, 2, 128).transpose(2, 0, 1).reshape(128, BOUT * 2)
    )

    if "nc" not in _CACHE:
        _CACHE["nc"] = _build_kernel()
    nc = _CACHE["nc"]

    in_maps = []
    for b in range(B):
        xt = np.ascontiguousarray(
            x[b].T.reshape(BIN, 2, 128, T).transpose(0, 2, 1, 3)
        )
        in_maps.append(
            {"xt": xt, "vt_w": vt_w, "u_w": u_w, "s_w": s_w, "bias_w": bias_w}
        )

    res = run_bass_kernel_spmd(nc, in_maps, core_ids=list(range(B)))

    out = np.empty((B, T, D), dtype=np.float32)
    for b in range(B):
        o = res.results[b]["ot"]  # (o, p, h, t)
        out[b] = o.transpose(3, 0, 2, 1).reshape(T, D)
    return out
